# revision 19
# baseline (speedup 1.0000x reference)
"""CosineContrastiveLoss_NoExp kernel for 8 trn2 NeuronCores.

Strategy: shard the HW (=512*512) axis across the 8 cores; each core gets a
contiguous 32768-element slice of every sample, laid out on host as
[p=128 partitions, chunk c, b, q] so device DMAs are fully contiguous.
The binary mask is shipped as uint8 (4x less HBM traffic) and consumed
directly by mixed-dtype multiplies.  Every quantity the loss needs is a sum
over HW of products of per-sample planes, so each core computes partial
sums over its slice and the host combines the 8 tiny [65,65] partials.

Per-core SBUF layout (chunk-major so chunk DMAs are contiguous):
  A  [128, 65*QTOT] f32, per chunk units: 0..31 sq1(b)=sigmoid(in1)^2,
        32..63 t1(b)=mask*sq1, 64 ones
  Bt [128, 65*QTOT] f32, per chunk units: 0..31 sq2(b)=sigmoid(in2)^2,
        32..63 t2(b)=mask*sq2, 64 ones
  MK [128, B*QTOT] u8: mask, b-major (single full DMA)
One accumulating PE series over q (256 matmuls, lhsT=A-chunk[:,:,qh]
[128,65], rhs=Bt-chunk[:,:,qh] [128,65]) yields in PSUM[65,65]
(mask binary -> mask^2 = mask):
  out[b,d]       = sum sq1[b]*sq2[d]          (gram G)
  out[b,64]      = sum sq1[b]                 (s1)
  out[64,d]      = sum sq2[d]                 (s2)
  out[32+b,b]    = sum mask*sq1*sq2           (pn)
  out[32+b,64]   = sum mask[b]*sq1[b]         (d1)
  out[64,32+d]   = sum mask[d]*sq2[d]         (d2)
The host reduces the 8 partials and evaluates the final scalar loss.
"""

import os

import numpy as np

B = 32
H = 512
W = 512
HWTOT = H * W          # 262144
NCORES = 8
P = 128
QTOT = HWTOT // (NCORES * P)   # 256 q per core
HWC = P * QTOT                 # 32768 per core
QSIZES = [32, 32, 32, 32, 32, 24, 24, 16, 16, 8, 4, 4]  # uneven: small tail chunks
assert sum(QSIZES) == QTOT
NCH = len(QSIZES)
M = 2 * B + 1                  # 65 stationary columns
N = 2 * B + 1                  # 65 moving columns

_CACHE = {}


def _build():
    import concourse.bacc as bacc
    import concourse.tile as tile
    import concourse.mybir as mybir

    f32 = mybir.dt.float32
    u8 = mybir.dt.uint8
    nc = bacc.Bacc("TRN2", target_bir_lowering=False, debug=False)
    in1 = nc.dram_tensor("in1", [P, B * QTOT], f32, kind="ExternalInput")
    in2 = nc.dram_tensor("in2", [P, B * QTOT], f32, kind="ExternalInput")
    mk8 = nc.dram_tensor("mk8", [P, B * QTOT], u8, kind="ExternalInput")
    out = nc.dram_tensor("out", [M, N], f32, kind="ExternalOutput")

    sig = mybir.ActivationFunctionType.Sigmoid

    with tile.TileContext(nc) as tc:
        with (
            tc.tile_pool(name="big", bufs=1) as big,
            tc.tile_pool(name="psp", bufs=1, space="PSUM") as psp,
            tc.tile_pool(name="outp", bufs=1) as outp,
        ):
            A = big.tile([P, M * QTOT], f32)
            Bt = big.tile([P, N * QTOT], f32)
            MK = big.tile([P, B, QTOT], u8)
            acc = psp.tile([M, N], f32)
            # warm the ACT sigmoid table while the first DMA is in flight
            warm = big.tile([1, 8], f32)
            nc.vector.memset(warm[:], 0.0)
            nc.scalar.activation(out=warm[:], in_=warm[:], func=sig)
            qoff = 0
            first = True
            for c, qs in enumerate(QSIZES):
                Av = A[:, M * qoff:M * (qoff + qs)].rearrange(
                    "p (u q) -> p u q", u=M)
                Bv = Bt[:, N * qoff:N * (qoff + qs)].rearrange(
                    "p (u q) -> p u q", u=N)
                nc.vector.memset(Av[:, 2 * B, :], 1.0)
                nc.vector.memset(Bv[:, 2 * B, :], 1.0)
                a_r = Av[:, 0:B, :]           # sigmoid(in1) -> sq1
                t1_r = Av[:, B:2 * B, :]      # t1 = mask*sq1
                b_r = Bv[:, 0:B, :]           # sigmoid(in2) -> sq2
                t2_r = Bv[:, B:2 * B, :]      # t2 = mask*sq2
                mk_r = MK[:, :, qoff:qoff + qs]
                sa = in1[:][:, B * qoff:B * (qoff + qs)].rearrange(
                    "p (u q) -> p u q", u=B)
                sb = in2[:][:, B * qoff:B * (qoff + qs)].rearrange(
                    "p (u q) -> p u q", u=B)
                nc.sync.dma_start(a_r, sa)
                nc.sync.dma_start(b_r, sb)
                if first:
                    # single full mask DMA (u8, contiguous 8KB runs)
                    nc.sync.dma_start(
                        MK[:], mk8[:].rearrange("p (u q) -> p u q", u=B))
                    first = False
                # sigmoids (ACT)
                nc.scalar.activation(out=a_r, in_=a_r, func=sig)
                nc.scalar.activation(out=b_r, in_=b_r, func=sig)
                # A-chain on DVE: sq1, then t1 = sq1 * mask(u8)
                nc.vector.tensor_mul(a_r, a_r, a_r)
                nc.vector.tensor_mul(t1_r, a_r, mk_r)
                # B-chain: sq2 on GPSIMD; t2 alternates DVE/GPSIMD for balance
                nc.gpsimd.tensor_mul(b_r, b_r, b_r)
                if c % 2 == 0:
                    nc.vector.tensor_mul(t2_r, b_r, mk_r)
                else:
                    nc.gpsimd.tensor_mul(t2_r, b_r, mk_r)
                for qh in range(qs):
                    q = qoff + qh
                    nc.tensor.matmul(
                        acc[:, :],
                        Av[:, :, qh],
                        Bv[:, :, qh],
                        start=(q == 0),
                        stop=(q == QTOT - 1),
                    )
                qoff += qs
            res = outp.tile([M, N], f32)
            nc.vector.tensor_copy(res[:], acc[:])
            nc.sync.dma_start(out[:], res[:])
    _batch_matmul_sem_updates(nc)
    nc.compile()
    return nc


def _batch_matmul_sem_updates(nc):
    """Tile emits a +1 sem-inc on every matmul, but the only consumers wait
    for the final value.  Strip the per-instruction updates (sequencer sem
    writes serialize at ~26-100ns each) and retarget the waiters to the
    reduced final count."""
    for blk in nc.m.functions[0].blocks:
        mms = [i for i in blk.instructions if type(i).__name__ == "InstMatmult"]
        if not mms:
            continue
        total = 0
        sem_id = None
        for i in mms:
            si = i.sync_info
            if si is None:
                continue
            for u in si.on_update:
                assert u.update_mode == "sem-inc"
                sem_id = u.id
                total += u.update_value
        kept = 0
        for i in mms[:-1]:
            si = i.sync_info
            if si is None:
                continue
            if len(si.on_wait) == 0 and len(si.on_update) == 1:
                i.sync_info = None
            else:
                kept += sum(u.update_value for u in si.on_update
                            if u.id == sem_id)
        kept += 1  # the last matmul keeps its +1
        for blk2 in nc.m.functions[0].blocks:
            for i in blk2.instructions:
                si = i.sync_info
                if si is None:
                    continue
                changed = False
                for w in si.on_wait:
                    if w.id == sem_id and w.wait_value == total:
                        w.wait_value = kept
                        changed = True
                if changed:
                    i.sync_info = si


def _get_nc():
    if "nc" not in _CACHE:
        _CACHE["nc"] = _build()
    return _CACHE["nc"]


def _shard_f32(x):
    """[B, HWTOT] f32 -> per-core [P, B*QTOT], per-chunk blocks (B, qs)."""
    v = x.reshape(B, NCORES, P, QTOT)
    parts = []
    qoff = 0
    for qs in QSIZES:
        blk = v[..., qoff:qoff + qs].transpose(1, 2, 0, 3)  # [NC, P, B, qs]
        parts.append(blk.reshape(NCORES, P, B * qs))
        qoff += qs
    out = np.ascontiguousarray(np.concatenate(parts, axis=2), dtype=np.float32)
    return [out[k] for k in range(NCORES)]


def _shard_mask_u8(mk):
    """mask -> per-core [P, B*QTOT] u8, b-major (no chunking)."""
    v = mk.reshape(B, NCORES, P, QTOT).transpose(1, 2, 0, 3)  # [NC, P, B, QTOT]
    out = np.ascontiguousarray(v).astype(np.uint8)
    return [out[k].reshape(P, B * QTOT) for k in range(NCORES)]


LAST_RESULT = None


def kernel(input1, input2, mask):
    from concourse.bass_utils import run_bass_kernel_spmd

    global LAST_RESULT
    x1 = np.asarray(input1, dtype=np.float32).reshape(B, HWTOT)
    x2 = np.asarray(input2, dtype=np.float32).reshape(B, HWTOT)
    mk = np.asarray(mask, dtype=np.float32).reshape(B, HWTOT)
    s1, s2, sm = _shard_f32(x1), _shard_f32(x2), _shard_mask_u8(mk)
    in_maps = [
        {"in1": s1[k], "in2": s2[k], "mk8": sm[k]} for k in range(NCORES)
    ]
    nc = _get_nc()
    trace = bool(int(os.environ.get("BASSKERNEL_TRACE", "0")))
    try:
        res = run_bass_kernel_spmd(
            nc, in_maps, core_ids=list(range(NCORES)), trace=trace,
        )
    except ModuleNotFoundError:
        # axon NTFF profile hook unavailable in this container
        res = run_bass_kernel_spmd(
            nc, in_maps, core_ids=list(range(NCORES)), trace=False,
        )
    LAST_RESULT = res

    Ms = np.zeros((M, N), dtype=np.float64)
    for r in res.results:
        Ms += np.asarray(r["out"], dtype=np.float64)

    G = Ms[0:B, 0:B]
    s1v = Ms[0:B, 2 * B]
    s2v = Ms[2 * B, 0:B]
    pn = np.diag(Ms[B:2 * B, 0:B])
    d1 = Ms[B:2 * B, 2 * B]
    d2 = Ms[2 * B, B:2 * B]

    sim_pos = np.sqrt(pn) / (np.sqrt(d1) * np.sqrt(d2))          # [B]
    sim = np.sqrt(G) / (np.sqrt(s1v)[:, None] * np.sqrt(s2v)[None, :])
    sim_neg = sim.sum(axis=1) - np.diag(sim)                      # [B]
    ratio = sim_pos[None, :] / (sim_pos[None, :] + sim_neg[:, None])
    loss = -np.log(ratio)
    return np.array(loss.mean(), dtype=np.float32)
